# revision 36
# baseline (speedup 1.0000x reference)
"""Single-head causal attention (B=8, T=2048, D=1024, H=128) on 8 TRN2 NeuronCores.

Sharding: one batch element per core (data-parallel over B).

v5 design (per core, bf16 inputs, fp32 PSUM accumulation):
  - packed weights w = [V|K|Q] d-tiles, DMA'd as three slices interleaved
    with the first x d-tiles; x^T first half streamed per d-tile, second
    half as quarter blocks ordered so the late projection groups unblock
    in need-order.
  - projections run as 512-col groups (g0..g3), each 8 d-steps of a V/K/Q
    matmul trio into three 1-bank PSUM accs. g0 tracks the HBM stream;
    g1 and chunk-3's Q run inline; the remaining groups are chopped into
    per-d closures and EMITTED AS FILLER between attention units, so the
    PE chews projection work exactly where the ACT-bound attention stretch
    would otherwise idle it, and the ACT exp stream (the attention-phase
    bottleneck at ~0.95ns/col) never waits on a monolithic proj pass.
  - attention per 512-wide q-chunk: two k-tiles per unit share a
    [128,1024] S PSUM tile; diagonal tiles write left-shifted so each
    unit's S area is contiguous -> ONE exp per unit (2c+2 ACT calls per
    chunk). Causal mask via DVE multiply on the first 128 cols of each
    diagonal region. Chunk order 0,1,3,2 with chunk-3 fed by the early Q
    pass. PV accumulates into a single PSUM bank; the O^T bf16 cast is
    emitted inline at body end (frees the bank), the denominator matmul
    one body later (its DVE-dependent wait never blocks the PE queue).
  - unnormalized O^T (bf16) + per-column sums (f32) DMA'd out; the host
    divides and transposes.
  - 40 warmup matmuls bridge the DMA lead-in so the HAM un-throttles the
    PE clock before real work starts and never re-throttles.
  - PSUM banks: S-ring 2x[128,1024]=4, vacc/kacc/qacc 3 (also host the
    V-transpose tiles and denominator rows), otp 1 -> exactly 8.
"""
import numpy as np

B, T, D, H = 8, 2048, 1024, 128
ND = D // 128      # 8 d-tiles
NTK = T // 128     # 16 k-tiles
NCH = T // 512     # 4 q-chunks
SCALE = float(H) ** -0.5

_CACHE = {}


def _build():
    import concourse.bass as bass  # noqa: F401
    from concourse import bacc
    import concourse.mybir as mybir
    import concourse.tile as tile
    from concourse.masks import make_identity

    f32 = mybir.dt.float32
    bf16 = mybir.dt.bfloat16

    nc = bacc.Bacc("TRN2", target_bir_lowering=False)
    xt_d = nc.dram_tensor("xt", (128, ND, T), bf16, kind="ExternalInput")
    # w[p, 8o+d, h]: o=0 V, o=1 K, o=2 Q
    w_d = nc.dram_tensor("w", (128, 3 * ND, H), bf16, kind="ExternalInput")
    ot_d = nc.dram_tensor("ot", (H, T), bf16, kind="ExternalOutput")
    sums_d = nc.dram_tensor("sums", (1, T), f32, kind="ExternalOutput")
    # chunk 2 is processed last: its denominator partials go to the host
    # unreduced, cutting the sums matmul + copy from the exposed tail
    pacc2_d = nc.dram_tensor("pacc2", (128, 512), bf16, kind="ExternalOutput")

    with tile.TileContext(nc) as tc:
        with (
            tc.tile_pool(name="sb", bufs=1) as sb,
            tc.tile_pool(name="ps", bufs=1, space="PSUM") as ps,
        ):
            # ---- constants ----
            warmsrc = sb.tile([128, 128], bf16, tag="warmsrc")
            nc.gpsimd.memset(warmsrc[:], 1.0)
            ident = sb.tile([128, 128], bf16, tag="ident")
            make_identity(nc, ident[:])
            tri32 = sb.tile([128, 128], f32, tag="tri32")
            nc.gpsimd.memset(tri32[:], 1.0)
            nc.gpsimd.affine_select(
                out=tri32[:], in_=tri32[:],
                compare_op=mybir.AluOpType.is_ge, fill=0.0,
                base=0, pattern=[[1, 128]], channel_multiplier=-1,
            )
            trimask = sb.tile([128, 128], bf16, tag="trimask")
            nc.vector.tensor_copy(trimask[:], tri32[:])
            ones_col = sb.tile([128, 1], bf16, tag="ones_col")
            nc.gpsimd.memset(ones_col[:], 1.0)
            warm = sb.tile([128, 1], bf16, tag="warm")
            nc.scalar.activation(warm[:], warmsrc[:, 0:1],
                                 mybir.ActivationFunctionType.Exp, scale=1.0)

            # ---- input DMA launches (need-order on one HW queue) ----
            w = sb.tile([128, 3 * ND, H], bf16, tag="w")
            xt = sb.tile([128, ND, T], bf16, tag="xt")
            nc.sync.dma_start(w[:, 0:8, :], w_d[:, 0:8, :])
            nc.sync.dma_start(xt[:, 0, 0:1024], xt_d[:, 0, 0:1024])
            nc.sync.dma_start(w[:, 8:16, :], w_d[:, 8:16, :])
            nc.sync.dma_start(xt[:, 1, 0:1024], xt_d[:, 1, 0:1024])
            nc.sync.dma_start(w[:, 16:24, :], w_d[:, 16:24, :])
            for d in range(2, ND):
                nc.sync.dma_start(xt[:, d, 0:1024], xt_d[:, d, 0:1024])
            nc.sync.dma_start(xt[:, 0:4, 1536:2048], xt_d[:, 0:4, 1536:2048])
            nc.sync.dma_start(xt[:, 4:8, 1536:2048], xt_d[:, 4:8, 1536:2048])
            nc.sync.dma_start(xt[:, 0:4, 1024:1536], xt_d[:, 0:4, 1024:1536])
            nc.sync.dma_start(xt[:, 4:8, 1024:1536], xt_d[:, 4:8, 1024:1536])

            # ---- PE warmup across the DMA lead-in ----
            wmm = ps.tile([128, 1024], f32, tag="ring", bufs=2, name="wmm")
            for i in range(40):
                nc.tensor.matmul(wmm[:, 0:128], warmsrc[:], warmsrc[:],
                                 start=(i == 0), stop=(i == 39))
            # second warmup chain: interleaved into the DMA-gated g0 group
            # so the HAM never sees an idle window during the stream-in
            wmm2 = ps.tile([128, 1024], f32, tag="ring", bufs=2, name="wmm2")
            _wmm2_n = [0]

            def wmm_fill(last=False):
                nc.tensor.matmul(wmm2[:, 0:128], warmsrc[:], warmsrc[:],
                                 start=(_wmm2_n[0] == 0), stop=last)
                _wmm2_n[0] += 1

            qt = sb.tile([128, T], bf16, tag="qt")
            kt = sb.tile([128, T], bf16, tag="kt")
            v = sb.tile([128, NTK, H], bf16, tag="v")
            sums_sb = sb.tile([1, T], f32, tag="sums_sb")

            OUTS = {"v": 0, "k": 1, "q": 2}
            ACCTAG = {"v": "vacc", "k": "kacc", "q": "qacc"}

            def group_accs(tlo, outs):
                return {o: ps.tile([128, 512], f32, tag=ACCTAG[o], bufs=1,
                                   name=f"{o}acc_{tlo}") for o in outs}

            def group_trio(tlo, accs, d, outs):
                for o in outs:
                    nc.tensor.matmul(
                        accs[o][:], w[:, 8 * OUTS[o] + d, :],
                        xt[:, d, tlo:tlo + 512],
                        start=(d == 0), stop=(d == ND - 1))

            def group_evac(tlo, accs, outs):
                with nc.allow_low_precision(reason="bf16 qkv"):
                    if "k" in outs:
                        # halves: the first 2 k-tiles unblock the next
                        # attention unit earlier
                        for hh in range(2):
                            nc.scalar.activation(
                                kt[:, tlo + 256 * hh:tlo + 256 * (hh + 1)],
                                accs["k"][:, 256 * hh:256 * (hh + 1)],
                                mybir.ActivationFunctionType.Copy)
                    if "q" in outs:
                        nc.vector.tensor_copy(qt[:, tlo:tlo + 512],
                                              accs["q"][:])
                    if "v" in outs:
                        vt = sb.tile([128, 512], bf16, tag="vt", bufs=2,
                                     name=f"vt_{tlo}")
                        nc.vector.tensor_copy(vt[:], accs["v"][:])
                        tp = ps.tile([128, 512], bf16, tag="vacc", bufs=1,
                                     name=f"tp_{tlo}")
                        for jj in range(4):
                            nc.tensor.transpose(
                                tp[:, jj * 128:(jj + 1) * 128],
                                vt[:, jj * 128:(jj + 1) * 128], ident[:])
                        nc.vector.tensor_copy(
                            v[:, tlo // 128:tlo // 128 + 4, :], tp[:])

            def group_inline(tlo, outs=("v", "k", "q"), wmm_fill_n=0):
                accs = group_accs(tlo, outs)
                for d in range(ND):
                    group_trio(tlo, accs, d, outs)
                    for i in range(wmm_fill_n):
                        wmm_fill(last=(d == ND - 1 and i == wmm_fill_n - 1))
                group_evac(tlo, accs, outs)

            def body(c, cast_on_act=False):
                nk = 4 * c + 4
                otp = ps.tile([128, 512], f32, tag="otacc", bufs=1,
                              name=f"otp{c}")
                pacc = sb.tile([128, 512], bf16, tag="pacc", bufs=4,
                               name=f"pacc{c}")
                units = [(2 * k, 2 * k + 1) for k in range(2 * c + 2)]

                def geom(j, prev_w):
                    lo = 128 * (j - 4 * c) if j >= 4 * c else 0
                    return prev_w, 512 - lo, lo

                def emit_su(u):
                    j0, j1 = units[u]
                    stp = ps.tile([128, 1024], f32, tag="ring", bufs=2,
                                  name=f"stp{c}_{u}")
                    pt = sb.tile([128, 1024], bf16, tag="pt", bufs=4,
                                 name=f"pt{c}_{u}")
                    base = 0
                    for j in (j0, j1):
                        base, wd, lo = geom(j, base)
                        nc.tensor.matmul(
                            stp[:, base:base + wd],
                            kt[:, j * 128:(j + 1) * 128],
                            qt[:, c * 512 + lo:(c + 1) * 512],
                            start=True, stop=True,
                        )
                        base += wd
                    nc.scalar.activation(
                        pt[:, 0:base], stp[:, 0:base],
                        mybir.ActivationFunctionType.Exp, scale=SCALE)
                    if j1 >= 4 * c:
                        base = 0
                        for j in (j0, j1):
                            base, wd, lo = geom(j, base)
                            nc.vector.tensor_mul(
                                pt[:, base:base + 128],
                                pt[:, base:base + 128], trimask[:])
                            base += wd
                    return pt

                def emit_pv(u, pt):
                    j0, j1 = units[u]
                    base = 0
                    for j in (j0, j1):
                        base, wd, lo = geom(j, base)
                        nc.tensor.matmul(
                            otp[:, lo:512], v[:, j, :], pt[:, base:base + wd],
                            start=(j == 0), stop=(j == nk - 1),
                        )
                        with nc.allow_low_precision(reason="bf16 denom"):
                            if j == 0:
                                nc.vector.tensor_copy(pacc[:], pt[:, 0:512])
                            else:
                                nc.vector.tensor_add(
                                    pacc[:, lo:512], pacc[:, lo:512],
                                    pt[:, base:base + wd])
                        base += wd

                U = len(units)
                pts = {}
                for u in range(min(2, U)):
                    pts[u] = emit_su(u)
                for u in range(U):
                    if u + 2 < U:
                        pts[u + 2] = emit_su(u + 2)
                    emit_pv(u, pts.pop(u))
                # inline: cast O^T to bf16 (frees the single otp bank) + DMA.
                # For the last body the cast runs on ACT (idle after the last
                # exp) so it doesn't serialize behind the DVE mask/add tail.
                ot_sb = sb.tile([128, 512], bf16, tag="otsb", bufs=2,
                                name=f"otsb{c}")
                with nc.allow_low_precision(reason="bf16 unnormalized out"):
                    if cast_on_act:
                        nc.scalar.activation(
                            ot_sb[:], otp[:],
                            mybir.ActivationFunctionType.Copy)
                    else:
                        nc.vector.tensor_copy(ot_sb[:], otp[:])
                nc.sync.dma_start(ot_d[:, c * 512:(c + 1) * 512], ot_sb[:])
                return pacc

            def tail_sums(c, pacc):
                """Denominator for chunk c, emitted one body late so its
                wait on the DVE pacc adds never blocks the PE queue."""
                sums = ps.tile([1, 512], f32, tag="kacc", bufs=1,
                               name=f"sums{c}")
                nc.tensor.matmul(sums[:], ones_col[:], pacc[:],
                                 start=True, stop=True)
                nc.vector.tensor_copy(sums_sb[:, c * 512:(c + 1) * 512],
                                      sums[:])

            # ---- schedule ----
            group_inline(0, wmm_fill_n=2)     # g0: DMA-gated, HAM kept busy
            group_inline(512)                 # g1
            # the rest of the projection is emitted at heavily DELAYED
            # priority: the scheduler weaves these matmuls into PE idle
            # slots of the ACT-bound attention stretch, and their evac
            # copies can never preempt the exp stream on ACT/DVE
            with tc.high_priority(offset=-1000000):
                group_inline(1536, outs=("q",))   # Q chunk 3 early
                # g2 (k-tiles 8-11 + Q chunk 2) before g3vk: body-3 units
                # 4-5 AND all of body-2 depend on it; only body-3's last
                # two units need k-tiles 12-15
                group_inline(1024)
                group_inline(1536, outs=("v", "k"))
            p0 = body(0)
            p1 = body(1)
            tail_sums(0, p0)
            p3 = body(3)
            tail_sums(1, p1)
            nc.sync.dma_start(sums_d[0, 0:1024], sums_sb[:, 0:1024])
            p2 = body(2, cast_on_act=True)
            # pulled into body-2's priority range so the scheduler slots the
            # chunk-3 denominator into b2's PE stream instead of the tail
            with tc.high_priority(offset=250):
                tail_sums(3, p3)
            nc.sync.dma_start(sums_d[0, 1536:2048], sums_sb[:, 1536:2048])
            nc.sync.dma_start(pacc2_d[:], p2[:])

    nc.compile()
    return nc


def _in_maps(x, W_Q, W_K, W_V):
    import ml_dtypes

    bf16 = ml_dtypes.bfloat16

    def warr(W):
        return np.asarray(W, np.float32).reshape(ND, 128, H).transpose(1, 0, 2)

    wr = np.ascontiguousarray(
        np.concatenate([warr(W_V), warr(W_K), warr(W_Q)], axis=1)
    ).astype(bf16)
    x = np.asarray(x, np.float32)
    return [
        {"xt": np.ascontiguousarray(
            x[b].T.reshape(ND, 128, T).transpose(1, 0, 2)).astype(bf16),
         "w": wr}
        for b in range(B)
    ]


def _run(inputs, **kw):
    from concourse import bass_utils

    if "nc" not in _CACHE:
        _CACHE["nc"] = _build()
    return bass_utils.run_bass_kernel_spmd(
        _CACHE["nc"], _in_maps(**inputs), core_ids=list(range(B)), **kw)


def kernel(x, W_Q, W_K, W_V):
    res = _run({"x": x, "W_Q": W_Q, "W_K": W_K, "W_V": W_V})
    out = np.empty((B, T, H), np.float32)
    for b in range(B):
        ot = np.asarray(res.results[b]["ot"], np.float32)   # [H, T]
        s = np.asarray(res.results[b]["sums"], np.float32)  # [1, T]
        # chunk 2's denominator comes back as unreduced bf16 partials
        p2 = np.asarray(res.results[b]["pacc2"], np.float32)  # [128, 512]
        s = s.copy()
        s[0, 1024:1536] = p2.sum(axis=0)
        out[b] = (ot / s).T
    return out
